# revision 6
# baseline (speedup 1.0000x reference)
"""Trainium2 Bass kernel for nn_AttentionTorch_77833397338547.

Computation (per batch b):
  K = keys[b,:,0,:]      [C=2048, S=1024]   (C = 16 heads x 128 head_dim)
  per head h (rows h*128:(h+1)*128 of the channel dim):
    scores[k, q] = (1/sqrt(128)) * K_h^T @ Q_h          [1024, 1024]
    P = softmax_k(scores + mask_bias)
    hid_h[d, q]  = V_h @ P                              [128, 1024]
  out[o, q] = sum_c w_out[o, c] * hid[c, q]             [2048, 1024]

Sharding: 8 cores = (batch b in 0..3) x (query half qh in 0..1).
Each core computes the full attention + out_proj for its (b, q-slice).
No cross-core communication is needed because out_proj only mixes
channels, which stay local to a core.

v3: all matmul operands stream as fp16 (1 PE cycle/row vs 2 for f32r
=> ~2x PE throughput; fp16 keeps rel err ~6e-4, far inside the 2e-2
gate).  Heads run in groups of 4 (two exp-batched pairs): one Exp
activation covers both heads of a pair's score chunk ([128, 1024]
per instruction; the mask bias is per key-chunk, head-independent,
so a single per-partition bias AP applies).  out_proj is interleaved
per group -- each group's 4 hid columns are immediately multiplied
into all 16 out-row-chunks (4 accumulating matmuls per chunk in
PSUM) and folded into an SBUF fp16 accumulator on DVE.  This keeps
the PE continuously busy (no serial ACT-bound attention phase
followed by a PE-only projection phase) and shrinks the
end-of-iteration tail to the last group's 64 matmuls + out DMA.
The softmax denominator is chunk-summed on DVE as a depth-3 tree
(fp16 SBUF operands hit the fast DVE modes) with a single
ones-matmul per head for the cross-partition sum + broadcast.
Output DMAs as fp16 (host converts to f32).
"""

import sys

sys.path.insert(0, "/opt/trn_rl_repo")

import numpy as np

B, C, S = 4, 2048, 1024
H, D = 16, 128          # heads x head_dim
QB = S // 2             # per-core query block = 512
KC = S // D             # key chunks per head = 8
OC = C // D             # out_proj row chunks = 16
GH = 4                  # heads per group (2 exp-batched pairs)
NG = H // GH            # groups = 4
N_CORES = 8
SCALE = 1.0 / np.sqrt(np.float32(D))
MASK_BIAS = np.float32(-60.0)

_BUILT = {}


def build_nc(repeat: int = 1):
    """Build + compile the per-core Bass program. Cached per config."""
    key = (repeat,)
    if key in _BUILT:
        return _BUILT[key]

    import concourse.bass as bass
    import concourse.mybir as mybir
    import concourse.tile as tile
    from concourse import bacc

    f32 = mybir.dt.float32
    f16 = mybir.dt.float16
    EXP = mybir.ActivationFunctionType.Exp

    nc = bacc.Bacc("TRN2", target_bir_lowering=False, debug=False,
                   num_devices=N_CORES)

    k_d = nc.dram_tensor("k_in", [C, S], f16, kind="ExternalInput")
    q_d = nc.dram_tensor("q_in", [C, QB], f16, kind="ExternalInput")
    v_d = nc.dram_tensor("v_in", [H, D, KC, D], f16, kind="ExternalInput")
    w_d = nc.dram_tensor("w_in", [H, D, OC, D], f16, kind="ExternalInput")
    bias_d = nc.dram_tensor("bias_in", [D, KC], f32, kind="ExternalInput")
    ones_d = nc.dram_tensor("ones_in", [D, D], f16, kind="ExternalInput")
    out_d = nc.dram_tensor("out", [C, QB], f16, kind="ExternalOutput")

    def body(tc):
        with (
            tc.tile_pool(name="const", bufs=1) as const,
            tc.tile_pool(name="kvq", bufs=2) as kvq,
            tc.tile_pool(name="ep", bufs=2) as ep,
            tc.tile_pool(name="tp", bufs=2) as tp,
            tc.tile_pool(name="hidp", bufs=2) as hidp,
            tc.tile_pool(name="wp", bufs=2) as wp,
            tc.tile_pool(name="rcp", bufs=2) as rcp,
            tc.tile_pool(name="accp", bufs=2) as accp,
            tc.tile_pool(name="scp", bufs=2, space="PSUM") as scp,
            tc.tile_pool(name="hpp", bufs=2, space="PSUM") as hpp,
            tc.tile_pool(name="aux", bufs=2, space="PSUM") as aux,
        ):
            ones_sb = const.tile([D, D], f16)
            bias_sb = const.tile([D, KC], f32)
            nc.sync.dma_start(ones_sb[:], ones_d[:])
            nc.sync.dma_start(bias_sb[:], bias_d[:])

            acc_out = accp.tile([D, OC, QB], f16)

            for g in range(NG):
                ks, qs, vs, ws = [], [], [], []
                for i in range(GH):
                    h = GH * g + i
                    k_sb = kvq.tile([D, S], f16, tag=f"k{i}")
                    q_sb = kvq.tile([D, QB], f16, tag=f"q{i}")
                    v_sb = kvq.tile([D, KC, D], f16, tag=f"v{i}")
                    w_sb = wp.tile([D, OC, D], f16, tag=f"w{i}")
                    nc.sync.dma_start(k_sb[:], k_d[h * D:(h + 1) * D, :])
                    nc.sync.dma_start(q_sb[:], q_d[h * D:(h + 1) * D, :])
                    nc.sync.dma_start(v_sb[:], v_d[h])
                    nc.sync.dma_start(w_sb[:], w_d[h])
                    ks.append(k_sb); qs.append(q_sb); vs.append(v_sb)
                    ws.append(w_sb)

                # scores + exp per pair: one ACT instr covers both heads'
                # chunk ([128, 1024]) -- bias is per key-chunk only
                es = []
                for p in range(2):
                    e2 = ep.tile([D, 2, KC, QB], f16, tag=f"e{p}")
                    for c in range(KC):
                        sc = scp.tile([D, 2, QB], f32)
                        for i in range(2):
                            nc.tensor.matmul(sc[:, i, :],
                                             ks[2 * p + i][:, c * D:(c + 1) * D],
                                             qs[2 * p + i][:],
                                             start=True, stop=True)
                        nc.scalar.activation(e2[:, :, c, :], sc[:], EXP,
                                             bias=bias_sb[:, c:c + 1], scale=1.0)
                    es.append(e2)

                hid4 = hidp.tile([D, GH, QB], f16)
                for i in range(GH):
                    e2, half = es[i // 2], i % 2
                    # denominator: depth-3 tree chunk-sum on DVE, then one
                    # ones-matmul for the cross-partition sum + broadcast
                    l1 = []
                    for a in range(4):
                        t = tp.tile([D, QB], f16, tag=f"l1_{a}")
                        nc.vector.tensor_add(t[:], e2[:, half, 2 * a, :],
                                             e2[:, half, 2 * a + 1, :])
                        l1.append(t)
                    l2 = []
                    for a in range(2):
                        t = tp.tile([D, QB], f16, tag=f"l2_{a}")
                        nc.vector.tensor_add(t[:], l1[2 * a][:], l1[2 * a + 1][:])
                        l2.append(t)
                    dacc = tp.tile([D, QB], f16, tag="dacc")
                    nc.vector.tensor_add(dacc[:], l2[0][:], l2[1][:])

                    hid_ps = hpp.tile([D, QB], f32)
                    for c in range(KC):
                        nc.tensor.matmul(hid_ps[:], vs[i][:, c, :],
                                         e2[:, half, c, :],
                                         start=(c == 0), stop=(c == KC - 1))
                    dn = aux.tile([D, QB], f32, tag="x")
                    nc.tensor.matmul(dn[:], ones_sb[:], dacc[:],
                                     start=True, stop=True)

                    rc = rcp.tile([D, QB], f32)
                    nc.vector.reciprocal(rc[:], dn[:])
                    nc.vector.tensor_mul(hid4[:, i, :], hid_ps[:], rc[:])

                # fold this group's 4 hid columns into the out accumulator
                for j in range(OC):
                    op = aux.tile([D, QB], f32, tag="x")
                    for i in range(GH):
                        nc.tensor.matmul(op[:], ws[i][:, j, :], hid4[:, i, :],
                                         start=(i == 0), stop=(i == GH - 1))
                    if g == 0:
                        nc.vector.tensor_copy(acc_out[:, j, :], op[:])
                    else:
                        nc.vector.tensor_add(acc_out[:, j, :],
                                             acc_out[:, j, :], op[:])
                    if g == NG - 1:
                        nc.sync.dma_start(out_d[j * D:(j + 1) * D, :],
                                          acc_out[:, j, :])

    with tile.TileContext(nc) as tc:
        if repeat == 1:
            body(tc)
        else:
            PE = mybir.EngineType.PE
            ACT = mybir.EngineType.Activation
            DVE = mybir.EngineType.DVE
            SP = mybir.EngineType.SP
            POOL = mybir.EngineType.Pool
            with tc.For_i(0, repeat, 1, hint_engines=(PE, ACT, DVE, SP, POOL)):
                body(tc)

    nc.compile()
    _BUILT[key] = nc
    return nc


def shard_inputs(keys, values, queries, attention_mask, w_out):
    """Host-side prep: slice per core and pre-layout for the device."""
    f16 = np.float16
    keys = np.asarray(keys, dtype=np.float32)
    values = np.asarray(values, dtype=np.float32)
    queries = np.asarray(queries, dtype=np.float32)
    mask = np.asarray(attention_mask)
    w_out = np.asarray(w_out, dtype=np.float32)

    # w_host[h, p, j, o] = w_out[j*128+o, h*128+p]; shared by all cores
    w_host = np.ascontiguousarray(
        w_out.reshape(OC, D, H, D).transpose(2, 3, 0, 1)).astype(f16)
    ones = np.ones((D, D), dtype=f16)

    in_maps = []
    for core in range(N_CORES):
        b, qh = core // 2, core % 2
        kb = np.ascontiguousarray(keys[b, :, 0, :]).astype(f16)      # [C, S]
        qb = (np.ascontiguousarray(
            queries[b, :, 0, qh * QB:(qh + 1) * QB]) * SCALE).astype(f16)
        # v_host[h, p, c, d] = values[b, h*128+d, 0, c*128+p]
        vb = np.ascontiguousarray(
            values[b, :, 0, :].reshape(H, D, KC, D).transpose(0, 3, 2, 1)
        ).astype(f16)
        bias = np.where(mask[b], np.float32(0.0), MASK_BIAS).astype(np.float32)
        bias = np.ascontiguousarray(bias.reshape(KC, D).T)      # [D, KC]
        in_maps.append({
            "k_in": kb, "q_in": qb, "v_in": vb,
            "w_in": w_host, "bias_in": bias, "ones_in": ones,
        })
    return in_maps


def kernel(keys, values, queries, attention_mask, w_out):
    from concourse.bass_utils import run_bass_kernel_spmd

    nc = build_nc(repeat=1)
    in_maps = shard_inputs(keys, values, queries, attention_mask, w_out)
    res = run_bass_kernel_spmd(nc, in_maps, list(range(N_CORES)))

    out = np.empty((B, C, 1, S), dtype=np.float32)
    for core in range(N_CORES):
        b, qh = core // 2, core % 2
        out[b, :, 0, qh * QB:(qh + 1) * QB] = res.results[core]["out"]
    return out


# revision 12
# speedup vs baseline: 1.0834x; 1.0834x over previous
"""Trainium2 Bass kernel for nn_AttentionTorch_77833397338547.

Computation (per batch b):
  K = keys[b,:,0,:]      [C=2048, S=1024]   (C = 16 heads x 128 head_dim)
  per head h (rows h*128:(h+1)*128 of the channel dim):
    scores[k, q] = (1/sqrt(128)) * K_h^T @ Q_h          [1024, 1024]
    P = softmax_k(scores + mask_bias)
    hid_h[d, q]  = V_h @ P                              [128, 1024]
  out[o, q] = sum_c w_out[o, c] * hid[c, q]             [2048, 1024]

Sharding: 8 cores = (batch b in 0..3) x (query half qh in 0..1).
Each core computes the full attention + out_proj for its (b, q-slice).
No cross-core communication is needed because out_proj only mixes
channels, which stay local to a core.

v3: all matmul operands stream as fp16 (1 PE cycle/row vs 2 for f32r
=> ~2x PE throughput; fp16 keeps rel err ~6e-4, far inside the 2e-2
gate).  Heads run in groups of 4 (two exp-batched pairs): one Exp
activation covers both heads of a pair's score chunk ([128, 1024]
per instruction; the mask bias is per key-chunk, head-independent,
so a single per-partition bias AP applies).  out_proj is interleaved
per group -- each group's 4 hid columns are immediately multiplied
into all 16 out-row-chunks (4 accumulating matmuls per chunk in
PSUM) and folded into an SBUF fp16 accumulator on DVE.  This keeps
the PE continuously busy (no serial ACT-bound attention phase
followed by a PE-only projection phase) and shrinks the
end-of-iteration tail to the last group's 64 matmuls + out DMA.
The softmax denominator is chunk-summed on DVE as a depth-3 tree
(fp16 SBUF operands hit the fast DVE modes) with a single
ones-matmul per head for the cross-partition sum + broadcast.
Output DMAs as fp16 (host converts to f32).
"""

import sys

sys.path.insert(0, "/opt/trn_rl_repo")

import numpy as np

B, C, S = 4, 2048, 1024
H, D = 16, 128          # heads x head_dim
QB = S // 2             # per-core query block = 512
KC = S // D             # key chunks per head = 8
OC = C // D             # out_proj row chunks = 16
GH = 4                  # heads per group (2 exp-batched pairs)
NG = H // GH            # groups = 4
N_CORES = 8
SCALE = 1.0 / np.sqrt(np.float32(D))
MASK_BIAS = np.float32(-60.0)

_BUILT = {}

# Staggered semaphore reset in the repeat loop: the 4 head-groups become
# pipeline stages, letting fast engines (DMA) run one stage ahead across
# the loop back-edge instead of hitting an all-engine barrier per
# iteration.
STAGGER = True


def build_nc(repeat: int = 1, stagger: bool = STAGGER):
    """Build + compile the per-core Bass program. Cached per config."""
    key = (repeat, stagger)
    if key in _BUILT:
        return _BUILT[key]

    import concourse.bass as bass
    import concourse.mybir as mybir
    import concourse.tile as tile
    from concourse import bacc

    f32 = mybir.dt.float32
    f16 = mybir.dt.float16
    EXP = mybir.ActivationFunctionType.Exp

    nc = bacc.Bacc("TRN2", target_bir_lowering=False, debug=False,
                   num_devices=N_CORES)

    k_d = nc.dram_tensor("k_in", [C, S], f16, kind="ExternalInput")
    q_d = nc.dram_tensor("q_in", [C, QB], f16, kind="ExternalInput")
    v_d = nc.dram_tensor("v_in", [H, D, KC, D], f16, kind="ExternalInput")
    w_d = nc.dram_tensor("w_in", [H, D, OC, D], f16, kind="ExternalInput")
    bias_d = nc.dram_tensor("bias_in", [D, KC], f32, kind="ExternalInput")
    ones_d = nc.dram_tensor("ones_in", [D, D], f16, kind="ExternalInput")
    out_d = nc.dram_tensor("out", [C, QB], f16, kind="ExternalOutput")

    def body(tc, staged=False):
        with (
            tc.tile_pool(name="const", bufs=1) as const,
            tc.tile_pool(name="kvq", bufs=2) as kvq,
            tc.tile_pool(name="ep", bufs=2) as ep,
            tc.tile_pool(name="tp", bufs=2) as tp,
            tc.tile_pool(name="hidp", bufs=2) as hidp,
            tc.tile_pool(name="wp", bufs=2) as wp,
            tc.tile_pool(name="rcp", bufs=2) as rcp,
            tc.tile_pool(name="accp", bufs=2) as accp,
            tc.tile_pool(name="scp", bufs=2, space="PSUM") as scp,
            tc.tile_pool(name="hpp", bufs=2, space="PSUM") as hpp,
            tc.tile_pool(name="aux", bufs=2, space="PSUM") as aux,
        ):
            ones_sb = const.tile([D, D], f16)
            bias_sb = const.tile([D, KC], f32)
            nc.sync.dma_start(ones_sb[:], ones_d[:])
            nc.sync.dma_start(bias_sb[:], bias_d[:])

            acc_out = accp.tile([D, OC, QB], f16)

            for g in range(NG):
                ks, qs, vs, ws = [], [], [], []
                for i in range(GH):
                    h = GH * g + i
                    k_sb = kvq.tile([D, S], f16, tag=f"k{i}")
                    q_sb = kvq.tile([D, QB], f16, tag=f"q{i}")
                    v_sb = kvq.tile([D, KC, D], f16, tag=f"v{i}")
                    w_sb = wp.tile([D, OC, D], f16, tag=f"w{i}")
                    nc.sync.dma_start(k_sb[:], k_d[h * D:(h + 1) * D, :])
                    nc.sync.dma_start(q_sb[:], q_d[h * D:(h + 1) * D, :])
                    nc.sync.dma_start(v_sb[:], v_d[h])
                    nc.sync.dma_start(w_sb[:], w_d[h])
                    ks.append(k_sb); qs.append(q_sb); vs.append(v_sb)
                    ws.append(w_sb)

                # scores + exp per pair: one ACT instr covers both heads'
                # chunk ([128, 1024]) -- bias is per key-chunk only
                es = []
                for p in range(2):
                    e2 = ep.tile([D, 2, KC, QB], f16, tag=f"e{p}")
                    for c in range(KC):
                        sc = scp.tile([D, 2, QB], f32)
                        for i in range(2):
                            nc.tensor.matmul(sc[:, i, :],
                                             ks[2 * p + i][:, c * D:(c + 1) * D],
                                             qs[2 * p + i][:],
                                             start=True, stop=True)
                        nc.scalar.activation(e2[:, :, c, :], sc[:], EXP,
                                             bias=bias_sb[:, c:c + 1], scale=1.0)
                    es.append(e2)

                hid4 = hidp.tile([D, GH, QB], f16)
                for i in range(GH):
                    e2, half = es[i // 2], i % 2
                    # denominator: strided tree chunk-sum on DVE (3 instrs),
                    # then one ones-matmul for the partition sum + broadcast
                    t1 = tp.tile([D, 4, QB], f16, tag="t1")
                    nc.vector.tensor_add(t1[:], e2[:, half, 0:4, :],
                                         e2[:, half, 4:8, :])
                    t2 = tp.tile([D, 2, QB], f16, tag="t2")
                    nc.vector.tensor_add(t2[:], t1[:, 0:2, :], t1[:, 2:4, :])
                    dacc = tp.tile([D, QB], f16, tag="dacc")
                    nc.vector.tensor_add(dacc[:], t2[:, 0, :], t2[:, 1, :])

                    hid_ps = hpp.tile([D, QB], f32)
                    for c in range(KC):
                        nc.tensor.matmul(hid_ps[:], vs[i][:, c, :],
                                         e2[:, half, c, :],
                                         start=(c == 0), stop=(c == KC - 1))
                    dn = aux.tile([D, QB], f32, tag="x")
                    nc.tensor.matmul(dn[:], ones_sb[:], dacc[:],
                                     start=True, stop=True)

                    rc = rcp.tile([D, QB], f32)
                    nc.vector.reciprocal(rc[:], dn[:])
                    nc.vector.tensor_mul(hid4[:, i, :], hid_ps[:], rc[:])

                # fold this group's 4 hid columns into the out accumulator:
                # PE -> PSUM op; a fast copy (alternating ACT/DVE) drains the
                # PSUM slot into fp16 staging; Pool (SBUF-only) accumulates.
                for j in range(OC):
                    op = aux.tile([D, QB], f32, tag="x")
                    for i in range(GH):
                        nc.tensor.matmul(op[:], ws[i][:, j, :], hid4[:, i, :],
                                         start=(i == 0), stop=(i == GH - 1))
                    stg = tp.tile([D, QB], f16, tag=f"stg{j % 2}")
                    if j % 2 == 0:
                        nc.scalar.copy(stg[:], op[:])
                    else:
                        nc.vector.tensor_copy(stg[:], op[:])
                    if g == 0:
                        nc.gpsimd.tensor_copy(acc_out[:, j, :], stg[:])
                    else:
                        nc.gpsimd.tensor_add(acc_out[:, j, :],
                                             acc_out[:, j, :], stg[:])
                    if g == NG - 1:
                        nc.sync.dma_start(out_d[j * D:(j + 1) * D, :],
                                          acc_out[:, j, :])

                if staged and g < NG - 1:
                    tc.stage_boundary()

    with tile.TileContext(nc) as tc:
        if repeat == 1:
            body(tc)
        else:
            PE = mybir.EngineType.PE
            ACT = mybir.EngineType.Activation
            DVE = mybir.EngineType.DVE
            SP = mybir.EngineType.SP
            POOL = mybir.EngineType.Pool
            with tc.For_i(0, repeat, 1, hint_engines=(PE, ACT, DVE, SP, POOL),
                          staggered_reset=stagger):
                body(tc, staged=stagger)

    nc.compile()
    _BUILT[key] = nc
    return nc


def shard_inputs(keys, values, queries, attention_mask, w_out):
    """Host-side prep: slice per core and pre-layout for the device."""
    f16 = np.float16
    keys = np.asarray(keys, dtype=np.float32)
    values = np.asarray(values, dtype=np.float32)
    queries = np.asarray(queries, dtype=np.float32)
    mask = np.asarray(attention_mask)
    w_out = np.asarray(w_out, dtype=np.float32)

    # w_host[h, p, j, o] = w_out[j*128+o, h*128+p]; shared by all cores
    w_host = np.ascontiguousarray(
        w_out.reshape(OC, D, H, D).transpose(2, 3, 0, 1)).astype(f16)
    ones = np.ones((D, D), dtype=f16)

    in_maps = []
    for core in range(N_CORES):
        b, qh = core // 2, core % 2
        kb = np.ascontiguousarray(keys[b, :, 0, :]).astype(f16)      # [C, S]
        qb = (np.ascontiguousarray(
            queries[b, :, 0, qh * QB:(qh + 1) * QB]) * SCALE).astype(f16)
        # v_host[h, p, c, d] = values[b, h*128+d, 0, c*128+p]
        vb = np.ascontiguousarray(
            values[b, :, 0, :].reshape(H, D, KC, D).transpose(0, 3, 2, 1)
        ).astype(f16)
        bias = np.where(mask[b], np.float32(0.0), MASK_BIAS).astype(np.float32)
        bias = np.ascontiguousarray(bias.reshape(KC, D).T)      # [D, KC]
        in_maps.append({
            "k_in": kb, "q_in": qb, "v_in": vb,
            "w_in": w_host, "bias_in": bias, "ones_in": ones,
        })
    return in_maps


def kernel(keys, values, queries, attention_mask, w_out):
    from concourse.bass_utils import run_bass_kernel_spmd

    nc = build_nc(repeat=1)
    in_maps = shard_inputs(keys, values, queries, attention_mask, w_out)
    res = run_bass_kernel_spmd(nc, in_maps, list(range(N_CORES)))

    out = np.empty((B, C, 1, S), dtype=np.float32)
    for core in range(N_CORES):
        b, qh = core // 2, core % 2
        out[b, :, 0, qh * QB:(qh + 1) * QB] = res.results[core]["out"]
    return out
